# revision 2
# baseline (speedup 1.0000x reference)
"""MoE segment-gated rank-1 LoRA projection for Trainium2 (8 NeuronCores).

Math: out[b,s,:] = sum_k topk_score[b,k] * SCALE * (x[b,s,:]@A[e_k]) * B[e_k]
Since gating is per-batch (segment level), this is, per batch b:
    H^T[e, t] = A[e, :] @ x[b]^T          (contract IN=1024)
    out[b]^T  = M2[b]^T.T @ H^T           (contract E=8)
where M2[b][e, :] = g[b, e] * SCALE * B[e, :], g zero for unselected experts.

Sharding: 8 cores <- (batch b = c//2, seq half h = c%2); each core owns 2048
tokens. Host does the tiny gating ([4,8] softmax/top-2) and packs x into a
chunk-major layout so each 256-token chunk is ONE contiguous 0.5MB DMA
(4KB per partition). Loads stream on the SP HWDGE ring, stores on the ACT
ring; per chunk: 8 accumulating matmuls (A@x), 8 rank-8 matmuls (M2@h),
8 DVE casts, 1 store DMA. The kernel is HBM-bound at ~358 GB/s/core.
"""

import numpy as np

import concourse.bass as bass
import concourse.tile as tile
from concourse import bacc, mybir
from concourse.bass_utils import run_bass_kernel_spmd

B, S, IN, OUT, E = 4, 4096, 1024, 1024, 8
TOPK = 2
SCALE = 512.0
TEMP = 1.0
N_CORES = 8
T = (B * S) // N_CORES          # 2048 tokens per core
P = 128
KT = IN // P                    # 8 contraction tiles
OTILES = OUT // P               # 8 output row-tiles

CHUNKS = [256] * 8              # tokens per pipeline chunk
assert sum(CHUNKS) == T
NCHUNK = len(CHUNKS)

_NC = None


def _build_bass():
    nc = bacc.Bacc()
    dt_mm = mybir.dt.bfloat16
    dt_out = mybir.dt.bfloat16
    # chunk-major packed layouts: 4KB contiguous per partition per chunk
    xT = nc.dram_tensor("xT", [P, KT * T], dt_mm, kind="ExternalInput")
    aT = nc.dram_tensor("aT", [P, KT * E], dt_mm, kind="ExternalInput")
    m2 = nc.dram_tensor("m2", [E, OUT], dt_mm, kind="ExternalInput")
    outT = nc.dram_tensor("outT", [P, OTILES * T], dt_out, kind="ExternalOutput")

    xoff, ooff = [], []
    base = 0
    for c in range(NCHUNK):
        xoff.append(KT * base)
        ooff.append(OTILES * base)
        base += CHUNKS[c]

    with tile.TileContext(nc) as tc:
        with (
            tc.tile_pool(name="consts", bufs=1) as consts,
            tc.tile_pool(name="xin", bufs=NCHUNK) as xin,
            tc.tile_pool(name="hbuf", bufs=3) as hbuf,
            tc.tile_pool(name="obuf", bufs=NCHUNK) as obuf,
            tc.tile_pool(name="psh", bufs=2, space="PSUM") as psh,
            tc.tile_pool(name="pso", bufs=5, space="PSUM") as pso,
            tc.tile_pool(name="warm", bufs=1, space="PSUM") as warm,
        ):
            # consts ride the otherwise-idle SWDGE (gpsimd) so the SP/ACT
            # HWDGE rings open with the first x load / first store
            a_sb = consts.tile([P, KT * E], dt_mm)
            nc.gpsimd.dma_start(a_sb[:], aT[:])
            m2_sb = consts.tile([E, OUT], dt_mm)
            nc.gpsimd.dma_start(m2_sb[:], m2[:])
            wsrc = consts.tile([P, 512], dt_mm)
            nc.vector.memset(wsrc[:], 0.0)
            wsink = consts.tile([P, 4], mybir.dt.float32)

            def emit_stage1(c):
                """load + matmul1 + h copy for chunk c; returns h tile."""
                CH = CHUNKS[c]
                xc = xin.tile([P, KT * CH], dt_mm, tag="x")
                nc.sync.dma_start(xc[:], xT[:, xoff[c]:xoff[c] + KT * CH])
                ph = psh.tile([E, CH], mybir.dt.float32)
                for k in range(KT):
                    nc.tensor.matmul(
                        ph[:],
                        a_sb[:, k * E:(k + 1) * E],
                        xc[:, k * CH:(k + 1) * CH],
                        start=(k == 0),
                        stop=(k == KT - 1),
                    )
                h = hbuf.tile([E, CH], dt_mm)
                nc.vector.tensor_copy(h[:], ph[:])
                # one full-array (128x128) matmul per chunk keeps the HAM
                # activity monitor fed: with only skinny (8-row/8-col) real
                # matmuls the clock gate throttles PE to 1.2GHz
                wt = warm.tile([P, 512], mybir.dt.float32)
                nc.tensor.matmul(wt[:], wsrc[:, 0:P], wsrc[:],
                                 start=True, stop=True)
                nc.vector.tensor_copy(wsink[:], wt[:, 0:4])
                return h

            def emit_stage2(c, h):
                """matmul2 + output casts + one store DMA for chunk c."""
                CH = CHUNKS[c]
                ob = obuf.tile([P, OTILES * CH], dt_out, tag="ob")
                for o in range(OTILES):
                    po = pso.tile([P, CH], mybir.dt.float32)
                    nc.tensor.matmul(
                        po[:],
                        m2_sb[:, o * P:(o + 1) * P],
                        h[:],
                        start=True,
                        stop=True,
                    )
                    nc.vector.tensor_copy(ob[:, o * CH:(o + 1) * CH], po[:])
                nc.scalar.dma_start(
                    outT[:, ooff[c]:ooff[c] + OTILES * CH], ob[:])

            # software pipeline: matmul1 of chunk c+1 is emitted before
            # matmul2 of chunk c, so the PE never stalls on the h copy
            hs = {0: emit_stage1(0)}
            for c in range(NCHUNK):
                if c + 1 < NCHUNK:
                    hs[c + 1] = emit_stage1(c + 1)
                emit_stage2(c, hs.pop(c))
    nc.compile()
    return nc


def _get_nc():
    global _NC
    if _NC is None:
        _NC = _build_bass()
    return _NC


def _host_gating(x, lora_A, lora_B, gate_w, gate_b):
    """Per-batch combined expert matrices M2[b] = sum of selected experts'
    score * SCALE * B rows (in the expert's row slot; rest zero)."""
    seg = np.asarray(x, np.float64).mean(axis=1)                    # [B, IN]
    logits = (seg @ np.asarray(gate_w, np.float64).T
              + np.asarray(gate_b, np.float64)) / TEMP              # [B, E]
    logits -= logits.max(axis=-1, keepdims=True)
    p = np.exp(logits)
    p /= p.sum(axis=-1, keepdims=True)
    top = np.argsort(-p, axis=-1, kind="stable")[:, :TOPK]          # [B, K]

    m2_all = np.zeros((B, E, OUT), np.float32)
    bcol = np.asarray(lora_B, np.float64)[:, :, 0]                  # [E, OUT]
    for b in range(B):
        for e in top[b]:
            m2_all[b, e, :] = (p[b, e] * SCALE) * bcol[e]
    return m2_all


def _pack_x(xc_f32, np_mm):
    """[T, IN] f32 -> [P, KT*T] chunk-major: block c is [P, KT, CH] with
    4KB contiguous per partition."""
    arr = xc_f32.astype(np_mm)
    blocks = []
    base = 0
    for CH in CHUNKS:
        blk = arr[base:base + CH].reshape(CH, KT, P).transpose(2, 1, 0)
        blocks.append(blk.reshape(P, KT * CH))
        base += CH
    return np.ascontiguousarray(np.concatenate(blocks, axis=1))


def _unpack_out(res):
    """[P, OTILES*T] chunk-major -> [T, OUT] f32."""
    out = np.empty((T, OUT), np.float32)
    base = 0
    off = 0
    for CH in CHUNKS:
        blk = res[:, off:off + OTILES * CH].reshape(P, OTILES, CH)
        out[base:base + CH] = blk.transpose(2, 1, 0).reshape(CH, OUT)
        base += CH
        off += OTILES * CH
    return out


def kernel(x, lora_A, lora_B, gate_w, gate_b):
    import ml_dtypes
    np_mm = ml_dtypes.bfloat16

    x = np.ascontiguousarray(np.asarray(x, np.float32))
    lora_A = np.asarray(lora_A, np.float32)
    lora_B = np.asarray(lora_B, np.float32)

    m2_all = _host_gating(x, lora_A, lora_B, gate_w, gate_b)

    # aT[p, k*E+e] = lora_A[e, 0, k*128+p]  (replicated on all cores)
    a_mat = lora_A[:, 0, :]                                          # [E, IN]
    aT = np.ascontiguousarray(
        a_mat.T.reshape(KT, P, E).transpose(1, 0, 2).reshape(P, KT * E)
    ).astype(np_mm)

    xr = x.reshape(N_CORES, T, IN)
    in_maps = []
    for c in range(N_CORES):
        in_maps.append({
            "xT": _pack_x(xr[c], np_mm),                             # [P, KT*T]
            "aT": aT,
            "m2": m2_all[c // 2].astype(np_mm),
        })

    res = run_bass_kernel_spmd(_get_nc(), in_maps, core_ids=list(range(N_CORES)))

    out = np.empty((N_CORES, T, OUT), np.float32)
    for c in range(N_CORES):
        out[c] = _unpack_out(res.results[c]["outT"])
    return out.reshape(B, S, OUT)


# revision 4
# speedup vs baseline: 1.1387x; 1.1387x over previous
"""MoE segment-gated rank-1 LoRA projection for Trainium2 (8 NeuronCores).

Math: out[b,s,:] = sum_k topk_score[b,k] * SCALE * (x[b,s,:]@A[e_k]) * B[e_k]
Since gating is per-batch (segment level), this is, per batch b:
    H^T[e, t] = A[e, :] @ x[b]^T          (contract IN=1024)
    out[b]^T  = M2[b]^T.T @ H^T           (contract E=8)
where M2[b][e, :] = g[b, e] * SCALE * B[e, :], g zero for unselected experts.

Sharding: 8 cores <- (batch b = c//2, seq half h = c%2); each core owns 2048
tokens. Host does the tiny gating ([4,8] softmax/top-2) and packs x into a
chunk-major layout so each 256-token chunk is ONE contiguous 0.5MB DMA
(4KB per partition). Loads stream on the SP HWDGE ring, stores on the ACT
ring.

Per chunk: matmul1 runs 4x column-tiled (col group j owns k-tiles {j, j+4},
partial h lands on PSUM partitions 32j..32j+7; the psh banks are zero-filled
once at start by a zero matmul so the other rows read 0.0), one DVE copy
drains the whole [128, CH] h-stack to SBUF, and matmul2 contracts K=128
against a host-built m2x whose rows 32j+e replicate m2[e] (the contraction
sums the 4 partial groups for free). The 8 output casts split DVE/ACT --
PSUM->SBUF drain is the second bottleneck after HBM (~358 GB/s/core).
"""

import numpy as np

import concourse.bass as bass
import concourse.tile as tile
from concourse import bacc, mybir
from concourse.bass_utils import run_bass_kernel_spmd

B, S, IN, OUT, E = 4, 4096, 1024, 1024, 8
TOPK = 2
SCALE = 512.0
TEMP = 1.0
N_CORES = 8
T = (B * S) // N_CORES          # 2048 tokens per core
P = 128
KT = IN // P                    # 8 contraction tiles
OTILES = OUT // P               # 8 output row-tiles
NGRP = 4                        # matmul1 column-tile groups
KPG = KT // NGRP                # k-tiles per group

CHUNKS = [256] * 8              # tokens per pipeline chunk
assert sum(CHUNKS) == T
NCHUNK = len(CHUNKS)

_NC = None


def _build_bass():
    nc = bacc.Bacc()
    dt_mm = mybir.dt.bfloat16
    dt_out = mybir.dt.bfloat16
    # chunk-major packed layouts: 4KB contiguous per partition per chunk
    xT = nc.dram_tensor("xT", [P, KT * T], dt_mm, kind="ExternalInput")
    aT = nc.dram_tensor("aT", [P, KT * E], dt_mm, kind="ExternalInput")
    m2x = nc.dram_tensor("m2x", [P, OUT], dt_mm, kind="ExternalInput")
    outT = nc.dram_tensor("outT", [P, OTILES * T], dt_out, kind="ExternalOutput")

    xoff, ooff = [], []
    base = 0
    for c in range(NCHUNK):
        xoff.append(KT * base)
        ooff.append(OTILES * base)
        base += CHUNKS[c]

    CHMAX = max(CHUNKS)

    with tile.TileContext(nc) as tc:
        with (
            tc.tile_pool(name="consts", bufs=1) as consts,
            tc.tile_pool(name="xin", bufs=NCHUNK) as xin,
            tc.tile_pool(name="hbuf", bufs=3) as hbuf,
            tc.tile_pool(name="obuf", bufs=NCHUNK) as obuf,
            tc.tile_pool(name="psh", bufs=2, space="PSUM") as psh,
            tc.tile_pool(name="pso", bufs=5, space="PSUM") as pso,
        ):
            # consts ride the otherwise-idle SWDGE (gpsimd) so the SP/ACT
            # HWDGE rings open with the first x load / first store
            a_sb = consts.tile([P, KT * E], dt_mm)
            nc.gpsimd.dma_start(a_sb[:], aT[:])
            m2_sb = consts.tile([P, OUT], dt_mm)
            nc.gpsimd.dma_start(m2_sb[:], m2x[:])
            wsrc = consts.tile([P, 512], dt_mm)
            nc.gpsimd.memset(wsrc[:], 0.0)

            # zero-fill both psh rotation slots so the partitions matmul1
            # never writes stay 0.0 (PSUM powers up with arbitrary bits;
            # NaN garbage would poison matmul2 despite its zero weights)
            for _ in range(2):
                z = psh.tile([P, CHMAX], mybir.dt.float32, tag="ph")
                nc.tensor.matmul(z[:], wsrc[:, 0:P], wsrc[:, 0:CHMAX],
                                 start=True, stop=True)

            def emit_stage1(c):
                """load + 4x col-tiled matmul1 + h drain for chunk c."""
                CH = CHUNKS[c]
                xc = xin.tile([P, KT * CH], dt_mm, tag="x")
                nc.sync.dma_start(xc[:], xT[:, xoff[c]:xoff[c] + KT * CH])
                ph = psh.tile([P, CHMAX], mybir.dt.float32, tag="ph")
                # wave i: groups 0..3 stream concurrently (distinct col
                # groups); group j accumulates k-tiles {j, j+NGRP} into
                # PSUM partitions 32j..32j+7
                for i in range(KPG):
                    for j in range(NGRP):
                        k = i * NGRP + j
                        nc.tensor.matmul(
                            ph[32 * j:32 * j + E, 0:CH],
                            a_sb[:, k * E:(k + 1) * E],
                            xc[:, k * CH:(k + 1) * CH],
                            start=(i == 0),
                            stop=(i == KPG - 1),
                            # explicit: base_partition auto-derive caps at 64
                            tile_position=(0, 32 * j),
                        )
                h = hbuf.tile([P, CH], dt_mm)
                nc.vector.tensor_copy(h[:], ph[:, 0:CH])
                return h

            def emit_stage2(c, h):
                """matmul2 (K=128 folds the 4 partial groups) + casts split
                DVE/ACT + one store DMA for chunk c."""
                CH = CHUNKS[c]
                ob = obuf.tile([P, OTILES * CH], dt_out, tag="ob")
                for o in range(OTILES):
                    po = pso.tile([P, CH], mybir.dt.float32)
                    nc.tensor.matmul(
                        po[:],
                        m2_sb[:, o * P:(o + 1) * P],
                        h[:],
                        start=True,
                        stop=True,
                    )
                    dst = ob[:, o * CH:(o + 1) * CH]
                    if o % 2 == 0:
                        nc.vector.tensor_copy(dst, po[:])
                    else:
                        nc.scalar.copy(dst, po[:])
                nc.scalar.dma_start(
                    outT[:, ooff[c]:ooff[c] + OTILES * CH], ob[:])

            # software pipeline: matmul1 of chunk c+1 is emitted before
            # matmul2 of chunk c, so the PE never stalls on the h drain
            hs = {0: emit_stage1(0)}
            for c in range(NCHUNK):
                if c + 1 < NCHUNK:
                    hs[c + 1] = emit_stage1(c + 1)
                emit_stage2(c, hs.pop(c))
    nc.compile()
    return nc


def _get_nc():
    global _NC
    if _NC is None:
        _NC = _build_bass()
    return _NC


def _host_gating(x, lora_A, lora_B, gate_w, gate_b):
    """Per-batch combined expert matrices M2[b] = sum of selected experts'
    score * SCALE * B rows (in the expert's row slot; rest zero)."""
    seg = np.asarray(x, np.float64).mean(axis=1)                    # [B, IN]
    logits = (seg @ np.asarray(gate_w, np.float64).T
              + np.asarray(gate_b, np.float64)) / TEMP              # [B, E]
    logits -= logits.max(axis=-1, keepdims=True)
    p = np.exp(logits)
    p /= p.sum(axis=-1, keepdims=True)
    top = np.argsort(-p, axis=-1, kind="stable")[:, :TOPK]          # [B, K]

    m2_all = np.zeros((B, E, OUT), np.float32)
    bcol = np.asarray(lora_B, np.float64)[:, :, 0]                  # [E, OUT]
    for b in range(B):
        for e in top[b]:
            m2_all[b, e, :] = (p[b, e] * SCALE) * bcol[e]
    return m2_all


def _pack_x(xc_f32, np_mm):
    """[T, IN] f32 -> [P, KT*T] chunk-major: block c is [P, KT, CH] with
    4KB contiguous per partition."""
    arr = xc_f32.astype(np_mm)
    blocks = []
    base = 0
    for CH in CHUNKS:
        blk = arr[base:base + CH].reshape(CH, KT, P).transpose(2, 1, 0)
        blocks.append(blk.reshape(P, KT * CH))
        base += CH
    return np.ascontiguousarray(np.concatenate(blocks, axis=1))


def _unpack_out(res):
    """[P, OTILES*T] chunk-major -> [T, OUT] f32."""
    out = np.empty((T, OUT), np.float32)
    base = 0
    off = 0
    for CH in CHUNKS:
        blk = res[:, off:off + OTILES * CH].reshape(P, OTILES, CH)
        out[base:base + CH] = blk.transpose(2, 1, 0).reshape(CH, OUT)
        base += CH
        off += OTILES * CH
    return out


def kernel(x, lora_A, lora_B, gate_w, gate_b):
    import ml_dtypes
    np_mm = ml_dtypes.bfloat16

    x = np.ascontiguousarray(np.asarray(x, np.float32))
    lora_A = np.asarray(lora_A, np.float32)
    lora_B = np.asarray(lora_B, np.float32)

    m2_all = _host_gating(x, lora_A, lora_B, gate_w, gate_b)

    # aT[p, k*E+e] = lora_A[e, 0, k*128+p]  (replicated on all cores)
    a_mat = lora_A[:, 0, :]                                          # [E, IN]
    aT = np.ascontiguousarray(
        a_mat.T.reshape(KT, P, E).transpose(1, 0, 2).reshape(P, KT * E)
    ).astype(np_mm)

    xr = x.reshape(N_CORES, T, IN)
    in_maps = []
    for c in range(N_CORES):
        # m2x rows 32j+e = m2[e]: matmul2's K=128 contraction then sums the
        # 4 column-tile partial h groups living at partitions 32j+e
        m2x = np.zeros((P, OUT), np.float32)
        for j in range(NGRP):
            m2x[32 * j:32 * j + E] = m2_all[c // 2]
        in_maps.append({
            "xT": _pack_x(xr[c], np_mm),                             # [P, KT*T]
            "aT": aT,
            "m2x": m2x.astype(np_mm),
        })

    res = run_bass_kernel_spmd(_get_nc(), in_maps, core_ids=list(range(N_CORES)))

    out = np.empty((N_CORES, T, OUT), np.float32)
    for c in range(N_CORES):
        out[c] = _unpack_out(res.results[c]["outT"])
    return out.reshape(B, S, OUT)


# revision 11
# speedup vs baseline: 1.2492x; 1.0971x over previous
"""MoE segment-gated rank-1 LoRA projection for Trainium2 (8 NeuronCores).

Math: out[b,s,:] = sum_k topk_score[b,k] * SCALE * (x[b,s,:]@A[e_k]) * B[e_k]
Since gating is per-batch (segment level), this is, per batch b:
    H^T[e, t] = A[e, :] @ x[b]^T          (contract IN=1024)
    out[b]^T  = M2[b]^T.T @ H^T           (contract E=8)
where M2[b][e, :] = g[b, e] * SCALE * B[e, :], g zero for unselected experts.

Sharding: 8 cores <- (batch b = c//2, seq half h = c%2); each core owns 2048
tokens. Host does the tiny gating ([4,8] softmax/top-2) and packs x into a
chunk-major layout so each 256-token chunk is ONE contiguous 0.5MB DMA
(4KB per partition). Loads stream on the SP HWDGE ring, stores on the ACT
ring.

Per chunk: matmul1 runs 4x column-tiled (col group j owns k-tiles {j, j+4},
partial h lands on PSUM partitions 32j..32j+7; the psh banks are zero-filled
once at start by a zero matmul so the other rows read 0.0), one DVE copy
drains the whole [128, CH] h-stack to SBUF, and matmul2 contracts K=128
against a host-built m2x whose rows 32j+e replicate m2[e] (the contraction
sums the 4 partial groups for free). The 8 output casts split DVE/ACT --
PSUM->SBUF drain is the second bottleneck after HBM (~358 GB/s/core).
"""

import numpy as np

import concourse.bass as bass
import concourse.tile as tile
from concourse import bacc, mybir
from concourse.bass_utils import run_bass_kernel_spmd

B, S, IN, OUT, E = 4, 4096, 1024, 1024, 8
TOPK = 2
SCALE = 512.0
TEMP = 1.0
N_CORES = 8
T = (B * S) // N_CORES          # 2048 tokens per core
P = 128
KT = IN // P                    # 8 contraction tiles
OTILES = OUT // P               # 8 output row-tiles
NGRP = 4                        # matmul1 column-tile groups
KPG = KT // NGRP                # k-tiles per group

CHUNKS = [128] + [256] * 7 + [128]   # small edges: faster pipe fill + drain
assert sum(CHUNKS) == T
NCHUNK = len(CHUNKS)
NPRIME = 6                      # PE warm-up matmuls before the pipeline

_NC = None


def _patch_walrus_max_sem_num(n=120):
    """Cap walrus's internal semaphore allocation; its codegen'd pre/post-
    amble resets every sem it may own, one instruction each, and that reset
    flood is ~6us of the measured kernel tail."""
    import concourse.bass_utils as _bu
    if getattr(_bu, "_max_sem_patched", None) == n:
        return
    orig = _bu.get_walrus_args

    def patched(arch, tmpdir, *, dve_root=None):
        return orig(arch, tmpdir, dve_root=dve_root) + [f"--max-sem-num={n}"]

    _bu.get_walrus_args = patched
    _bu._max_sem_patched = n


def _build_bass():
    nc = bacc.Bacc()
    dt_mm = mybir.dt.bfloat16
    dt_out = mybir.dt.bfloat16
    # chunk-major packed layouts: 4KB contiguous per partition per chunk
    xT = nc.dram_tensor("xT", [P, KT * T], dt_mm, kind="ExternalInput")
    aT = nc.dram_tensor("aT", [P, KT * E], dt_mm, kind="ExternalInput")
    m2x = nc.dram_tensor("m2x", [P, OUT], dt_mm, kind="ExternalInput")
    outT = nc.dram_tensor("outT", [P, OTILES * T], dt_out, kind="ExternalOutput")

    xoff, ooff = [], []
    base = 0
    for c in range(NCHUNK):
        xoff.append(KT * base)
        ooff.append(OTILES * base)
        base += CHUNKS[c]

    CHMAX = max(CHUNKS)

    with tile.TileContext(nc) as tc:
        with (
            tc.tile_pool(name="consts", bufs=1) as consts,
            tc.tile_pool(name="xin", bufs=NCHUNK) as xin,
            tc.tile_pool(name="hbuf", bufs=3) as hbuf,
            tc.tile_pool(name="obuf", bufs=NCHUNK) as obuf,
            tc.tile_pool(name="psh", bufs=2, space="PSUM") as psh,
            tc.tile_pool(name="pso", bufs=5, space="PSUM") as pso,
            tc.tile_pool(name="warm", bufs=1, space="PSUM") as warm,
        ):
            # consts ride the otherwise-idle SWDGE (gpsimd) so the SP/ACT
            # HWDGE rings open with the first x load / first store
            a_sb = consts.tile([P, KT * E], dt_mm)
            nc.gpsimd.dma_start(a_sb[:], aT[:])
            m2_sb = consts.tile([P, OUT], dt_mm)
            nc.gpsimd.dma_start(m2_sb[:], m2x[:])
            wsrc = consts.tile([P, 512], dt_mm)
            nc.gpsimd.memset(wsrc[:], 0.0)

            # zero-fill both psh rotation slots so the partitions matmul1
            # never writes stay 0.0 (PSUM powers up with arbitrary bits;
            # NaN garbage would poison matmul2 despite its zero weights)
            for _ in range(2):
                z = psh.tile([P, CHMAX], mybir.dt.float32, tag="ph")
                nc.tensor.matmul(z[:], wsrc[:, 0:P], wsrc[:, 0:CHMAX],
                                 start=True, stop=True)
            # PE warm-up: the HAM clock gate holds PE at 1.2GHz until it has
            # seen ~3.4us of sustained activity; burn that in while the
            # first x chunks stream in, and top the activity monitor up once
            # per chunk. These chain WAW on one PSUM tile (PE program order,
            # no cross-engine deps); a single DVE read at the end keeps DCE
            # off them.
            def emit_warm():
                wt = warm.tile([P, 512], mybir.dt.float32, tag="wt")
                nc.tensor.matmul(wt[:], wsrc[:, 0:P], wsrc[:],
                                 start=True, stop=True)
                return wt

            for _ in range(NPRIME):
                emit_warm()

            def emit_stage1(c):
                """load + 4x col-tiled matmul1 + h drain for chunk c."""
                CH = CHUNKS[c]
                xc = xin.tile([P, KT * CH], dt_mm, tag="x")
                nc.sync.dma_start(xc[:], xT[:, xoff[c]:xoff[c] + KT * CH])
                ph = psh.tile([P, CHMAX], mybir.dt.float32, tag="ph")
                # wave i: groups 0..3 stream concurrently (distinct col
                # groups); group j accumulates k-tiles {j, j+NGRP} into
                # PSUM partitions 32j..32j+7
                for i in range(KPG):
                    for j in range(NGRP):
                        k = i * NGRP + j
                        nc.tensor.matmul(
                            ph[32 * j:32 * j + E, 0:CH],
                            a_sb[:, k * E:(k + 1) * E],
                            xc[:, k * CH:(k + 1) * CH],
                            start=(i == 0),
                            stop=(i == KPG - 1),
                            # explicit: base_partition auto-derive caps at 64
                            tile_position=(0, 32 * j),
                        )
                h = hbuf.tile([P, CH], dt_mm)
                nc.vector.tensor_copy(h[:], ph[:, 0:CH])
                emit_warm()
                return h

            def emit_stage2(c, h):
                """matmul2 (K=128 folds the 4 partial groups) + casts split
                DVE/ACT + one store DMA for chunk c."""
                CH = CHUNKS[c]
                ob = obuf.tile([P, OTILES * CH], dt_out, tag="ob")
                for o in range(OTILES):
                    po = pso.tile([P, CH], mybir.dt.float32)
                    nc.tensor.matmul(
                        po[:],
                        m2_sb[:, o * P:(o + 1) * P],
                        h[:],
                        start=True,
                        stop=True,
                    )
                    dst = ob[:, o * CH:(o + 1) * CH]
                    if o % 2 == 0:
                        nc.vector.tensor_copy(dst, po[:])
                    else:
                        nc.scalar.copy(dst, po[:])
                # stores ride the SWDGE (gpsimd) ring: keeps ACT free for
                # its share of the PSUM->SBUF casts
                nc.gpsimd.dma_start(
                    outT[:, ooff[c]:ooff[c] + OTILES * CH], ob[:])

            # software pipeline: matmul1 of chunk c+1 is emitted before
            # matmul2 of chunk c, so the PE never stalls on the h drain
            hs = {0: emit_stage1(0)}
            for c in range(NCHUNK):
                if c + 1 < NCHUNK:
                    hs[c + 1] = emit_stage1(c + 1)
                emit_stage2(c, hs.pop(c))
            wlast = emit_warm()
            wsink = consts.tile([P, 4], mybir.dt.float32)
            nc.vector.tensor_copy(wsink[:], wlast[:, 0:4])
    nc.compile()
    return nc


def _get_nc():
    global _NC
    if _NC is None:
        _patch_walrus_max_sem_num()
        _NC = _build_bass()
    return _NC


def _host_gating(x, lora_A, lora_B, gate_w, gate_b):
    """Per-batch combined expert matrices M2[b] = sum of selected experts'
    score * SCALE * B rows (in the expert's row slot; rest zero)."""
    seg = np.asarray(x, np.float64).mean(axis=1)                    # [B, IN]
    logits = (seg @ np.asarray(gate_w, np.float64).T
              + np.asarray(gate_b, np.float64)) / TEMP              # [B, E]
    logits -= logits.max(axis=-1, keepdims=True)
    p = np.exp(logits)
    p /= p.sum(axis=-1, keepdims=True)
    top = np.argsort(-p, axis=-1, kind="stable")[:, :TOPK]          # [B, K]

    m2_all = np.zeros((B, E, OUT), np.float32)
    bcol = np.asarray(lora_B, np.float64)[:, :, 0]                  # [E, OUT]
    for b in range(B):
        for e in top[b]:
            m2_all[b, e, :] = (p[b, e] * SCALE) * bcol[e]
    return m2_all


def _pack_x(xc_f32, np_mm):
    """[T, IN] f32 -> [P, KT*T] chunk-major: block c is [P, KT, CH] with
    4KB contiguous per partition."""
    arr = xc_f32.astype(np_mm)
    blocks = []
    base = 0
    for CH in CHUNKS:
        blk = arr[base:base + CH].reshape(CH, KT, P).transpose(2, 1, 0)
        blocks.append(blk.reshape(P, KT * CH))
        base += CH
    return np.ascontiguousarray(np.concatenate(blocks, axis=1))


def _unpack_out(res):
    """[P, OTILES*T] chunk-major -> [T, OUT] f32."""
    out = np.empty((T, OUT), np.float32)
    base = 0
    off = 0
    for CH in CHUNKS:
        blk = res[:, off:off + OTILES * CH].reshape(P, OTILES, CH)
        out[base:base + CH] = blk.transpose(2, 1, 0).reshape(CH, OUT)
        base += CH
        off += OTILES * CH
    return out


def kernel(x, lora_A, lora_B, gate_w, gate_b):
    import ml_dtypes
    np_mm = ml_dtypes.bfloat16

    x = np.ascontiguousarray(np.asarray(x, np.float32))
    lora_A = np.asarray(lora_A, np.float32)
    lora_B = np.asarray(lora_B, np.float32)

    m2_all = _host_gating(x, lora_A, lora_B, gate_w, gate_b)

    # aT[p, k*E+e] = lora_A[e, 0, k*128+p]  (replicated on all cores)
    a_mat = lora_A[:, 0, :]                                          # [E, IN]
    aT = np.ascontiguousarray(
        a_mat.T.reshape(KT, P, E).transpose(1, 0, 2).reshape(P, KT * E)
    ).astype(np_mm)

    xr = x.reshape(N_CORES, T, IN)
    in_maps = []
    for c in range(N_CORES):
        # m2x rows 32j+e = m2[e]: matmul2's K=128 contraction then sums the
        # 4 column-tile partial h groups living at partitions 32j+e
        m2x = np.zeros((P, OUT), np.float32)
        for j in range(NGRP):
            m2x[32 * j:32 * j + E] = m2_all[c // 2]
        in_maps.append({
            "xT": _pack_x(xr[c], np_mm),                             # [P, KT*T]
            "aT": aT,
            "m2x": m2x.astype(np_mm),
        })

    res = run_bass_kernel_spmd(_get_nc(), in_maps, core_ids=list(range(N_CORES)))

    out = np.empty((N_CORES, T, OUT), np.float32)
    for c in range(N_CORES):
        out[c] = _unpack_out(res.results[c]["outT"])
    return out.reshape(B, S, OUT)


# revision 13
# speedup vs baseline: 1.2739x; 1.0198x over previous
"""MoE segment-gated rank-1 LoRA projection for Trainium2 (8 NeuronCores).

Math: out[b,s,:] = sum_k topk_score[b,k] * SCALE * (x[b,s,:]@A[e_k]) * B[e_k]
Since gating is per-batch (segment level), this is, per batch b:
    H^T[e, t] = A[e, :] @ x[b]^T          (contract IN=1024)
    out[b]^T  = M2[b]^T.T @ H^T           (contract E=8)
where M2[b][e, :] = g[b, e] * SCALE * B[e, :], g zero for unselected experts.

Sharding: 8 cores <- (batch b = c//2, seq half h = c%2); each core owns 2048
tokens. Host does the tiny gating ([4,8] softmax/top-2) and packs x into a
chunk-major layout so each 256-token chunk is ONE contiguous 0.5MB DMA
(4KB per partition). Loads stream on the SP HWDGE ring, stores on the ACT
ring.

Per chunk: matmul1 runs 4x column-tiled (col group j owns k-tiles {j, j+4},
partial h lands on PSUM partitions 32j..32j+7; the psh banks are zero-filled
once at start by a zero matmul so the other rows read 0.0), one DVE copy
drains the whole [128, CH] h-stack to SBUF, and matmul2 contracts K=128
against a host-built m2x whose rows 32j+e replicate m2[e] (the contraction
sums the 4 partial groups for free). The 8 output casts split DVE/ACT --
PSUM->SBUF drain is the second bottleneck after HBM (~358 GB/s/core).
"""

import numpy as np

import concourse.bass as bass
import concourse.tile as tile
from concourse import bacc, mybir
from concourse.bass_utils import run_bass_kernel_spmd

B, S, IN, OUT, E = 4, 4096, 1024, 1024, 8
TOPK = 2
SCALE = 512.0
TEMP = 1.0
N_CORES = 8
T = (B * S) // N_CORES          # 2048 tokens per core
P = 128
KT = IN // P                    # 8 contraction tiles
OTILES = OUT // P               # 8 output row-tiles
NGRP = 4                        # matmul1 column-tile groups
KPG = KT // NGRP                # k-tiles per group

# N=512 matmuls amortize the ~50ns/matmul dispatch overhead 2x vs N=256 and
# match the dense-stream pattern that reliably flips the PE HAM clock gate
# to 2.4GHz; small edge chunks speed pipeline fill + drain.
CHUNKS = [128, 256, 512, 512, 512, 128]
assert sum(CHUNKS) == T
NCHUNK = len(CHUNKS)
NPRIME = 10                     # PE warm-up matmuls before the pipeline

_NC = None


def _build_bass():
    nc = bacc.Bacc()
    dt_mm = mybir.dt.bfloat16
    dt_out = mybir.dt.bfloat16
    # chunk-major packed layouts: 4KB contiguous per partition per chunk
    xT = nc.dram_tensor("xT", [P, KT * T], dt_mm, kind="ExternalInput")
    aT = nc.dram_tensor("aT", [P, KT * E], dt_mm, kind="ExternalInput")
    m2x = nc.dram_tensor("m2x", [P, OUT], dt_mm, kind="ExternalInput")
    outT = nc.dram_tensor("outT", [P, OTILES * T], dt_out, kind="ExternalOutput")

    xoff, ooff = [], []
    base = 0
    for c in range(NCHUNK):
        xoff.append(KT * base)
        ooff.append(OTILES * base)
        base += CHUNKS[c]

    CHMAX = max(CHUNKS)

    with tile.TileContext(nc) as tc:
        with (
            tc.tile_pool(name="consts", bufs=1) as consts,
            tc.tile_pool(name="xin", bufs=NCHUNK) as xin,
            tc.tile_pool(name="hbuf", bufs=3) as hbuf,
            tc.tile_pool(name="obuf", bufs=NCHUNK) as obuf,
            tc.tile_pool(name="psh", bufs=2, space="PSUM") as psh,
            tc.tile_pool(name="pso", bufs=5, space="PSUM") as pso,
            tc.tile_pool(name="warm", bufs=1, space="PSUM") as warm,
        ):
            # consts ride the otherwise-idle SWDGE (gpsimd) so the SP/ACT
            # HWDGE rings open with the first x load / first store
            a_sb = consts.tile([P, KT * E], dt_mm)
            nc.gpsimd.dma_start(a_sb[:], aT[:])
            m2_sb = consts.tile([P, OUT], dt_mm)
            nc.gpsimd.dma_start(m2_sb[:], m2x[:])
            wsrc = consts.tile([P, 512], dt_mm)
            nc.gpsimd.memset(wsrc[:], 0.0)

            # zero-fill both psh rotation slots so the partitions matmul1
            # never writes stay 0.0 (PSUM powers up with arbitrary bits;
            # NaN garbage would poison matmul2 despite its zero weights)
            for _ in range(2):
                z = psh.tile([P, CHMAX], mybir.dt.float32, tag="ph")
                nc.tensor.matmul(z[:], wsrc[:, 0:P], wsrc[:, 0:CHMAX],
                                 start=True, stop=True)
            # PE warm-up: the HAM clock gate holds PE at 1.2GHz until it has
            # seen ~3.4us of sustained activity; burn that in while the
            # first x chunks stream in, and top the activity monitor up once
            # per chunk. These chain WAW on one PSUM tile (PE program order,
            # no cross-engine deps); a single DVE read at the end keeps DCE
            # off them.
            def emit_warm():
                wt = warm.tile([P, 512], mybir.dt.float32, tag="wt")
                nc.tensor.matmul(wt[:], wsrc[:, 0:P], wsrc[:],
                                 start=True, stop=True)
                return wt

            for _ in range(NPRIME):
                emit_warm()

            def emit_stage1(c):
                """load + 4x col-tiled matmul1 + h drain for chunk c."""
                CH = CHUNKS[c]
                xc = xin.tile([P, KT * CH], dt_mm, tag="x")
                nc.sync.dma_start(xc[:], xT[:, xoff[c]:xoff[c] + KT * CH])
                ph = psh.tile([P, CHMAX], mybir.dt.float32, tag="ph")
                # wave i: groups 0..3 stream concurrently (distinct col
                # groups); group j accumulates k-tiles {j, j+NGRP} into
                # PSUM partitions 32j..32j+7
                for i in range(KPG):
                    for j in range(NGRP):
                        k = i * NGRP + j
                        nc.tensor.matmul(
                            ph[32 * j:32 * j + E, 0:CH],
                            a_sb[:, k * E:(k + 1) * E],
                            xc[:, k * CH:(k + 1) * CH],
                            start=(i == 0),
                            stop=(i == KPG - 1),
                            # explicit: base_partition auto-derive caps at 64
                            tile_position=(0, 32 * j),
                        )
                h = hbuf.tile([P, CH], dt_mm)
                nc.vector.tensor_copy(h[:], ph[:, 0:CH])
                emit_warm()
                return h

            def emit_stage2(c, h):
                """matmul2 (K=128 folds the 4 partial groups) + casts split
                DVE/ACT + one store DMA for chunk c."""
                CH = CHUNKS[c]
                ob = obuf.tile([P, OTILES * CH], dt_out, tag="ob")
                for o in range(OTILES):
                    po = pso.tile([P, CH], mybir.dt.float32)
                    nc.tensor.matmul(
                        po[:],
                        m2_sb[:, o * P:(o + 1) * P],
                        h[:],
                        start=True,
                        stop=True,
                    )
                    dst = ob[:, o * CH:(o + 1) * CH]
                    if o % 2 == 0:
                        nc.vector.tensor_copy(dst, po[:])
                    else:
                        nc.scalar.copy(dst, po[:])
                # stores ride the SWDGE (gpsimd) ring: keeps ACT free for
                # its share of the PSUM->SBUF casts
                nc.gpsimd.dma_start(
                    outT[:, ooff[c]:ooff[c] + OTILES * CH], ob[:])

            # software pipeline: matmul1 of chunk c+1 is emitted before
            # matmul2 of chunk c, so the PE never stalls on the h drain
            hs = {0: emit_stage1(0)}
            for c in range(NCHUNK):
                if c + 1 < NCHUNK:
                    hs[c + 1] = emit_stage1(c + 1)
                emit_stage2(c, hs.pop(c))
            wlast = emit_warm()
            wsink = consts.tile([P, 4], mybir.dt.float32)
            nc.vector.tensor_copy(wsink[:], wlast[:, 0:4])
    nc.compile()
    return nc


def _get_nc():
    global _NC
    if _NC is None:
        _NC = _build_bass()
    return _NC


def _host_gating(x, lora_A, lora_B, gate_w, gate_b):
    """Per-batch combined expert matrices M2[b] = sum of selected experts'
    score * SCALE * B rows (in the expert's row slot; rest zero)."""
    seg = np.asarray(x, np.float64).mean(axis=1)                    # [B, IN]
    logits = (seg @ np.asarray(gate_w, np.float64).T
              + np.asarray(gate_b, np.float64)) / TEMP              # [B, E]
    logits -= logits.max(axis=-1, keepdims=True)
    p = np.exp(logits)
    p /= p.sum(axis=-1, keepdims=True)
    top = np.argsort(-p, axis=-1, kind="stable")[:, :TOPK]          # [B, K]

    m2_all = np.zeros((B, E, OUT), np.float32)
    bcol = np.asarray(lora_B, np.float64)[:, :, 0]                  # [E, OUT]
    for b in range(B):
        for e in top[b]:
            m2_all[b, e, :] = (p[b, e] * SCALE) * bcol[e]
    return m2_all


def _pack_x(xc_f32, np_mm):
    """[T, IN] f32 -> [P, KT*T] chunk-major: block c is [P, KT, CH] with
    4KB contiguous per partition."""
    arr = xc_f32.astype(np_mm)
    blocks = []
    base = 0
    for CH in CHUNKS:
        blk = arr[base:base + CH].reshape(CH, KT, P).transpose(2, 1, 0)
        blocks.append(blk.reshape(P, KT * CH))
        base += CH
    return np.ascontiguousarray(np.concatenate(blocks, axis=1))


def _unpack_out(res):
    """[P, OTILES*T] chunk-major -> [T, OUT] f32."""
    out = np.empty((T, OUT), np.float32)
    base = 0
    off = 0
    for CH in CHUNKS:
        blk = res[:, off:off + OTILES * CH].reshape(P, OTILES, CH)
        out[base:base + CH] = blk.transpose(2, 1, 0).reshape(CH, OUT)
        base += CH
        off += OTILES * CH
    return out


def kernel(x, lora_A, lora_B, gate_w, gate_b):
    import ml_dtypes
    np_mm = ml_dtypes.bfloat16

    x = np.ascontiguousarray(np.asarray(x, np.float32))
    lora_A = np.asarray(lora_A, np.float32)
    lora_B = np.asarray(lora_B, np.float32)

    m2_all = _host_gating(x, lora_A, lora_B, gate_w, gate_b)

    # aT[p, k*E+e] = lora_A[e, 0, k*128+p]  (replicated on all cores)
    a_mat = lora_A[:, 0, :]                                          # [E, IN]
    aT = np.ascontiguousarray(
        a_mat.T.reshape(KT, P, E).transpose(1, 0, 2).reshape(P, KT * E)
    ).astype(np_mm)

    xr = x.reshape(N_CORES, T, IN)
    in_maps = []
    for c in range(N_CORES):
        # m2x rows 32j+e = m2[e]: matmul2's K=128 contraction then sums the
        # 4 column-tile partial h groups living at partitions 32j+e
        m2x = np.zeros((P, OUT), np.float32)
        for j in range(NGRP):
            m2x[32 * j:32 * j + E] = m2_all[c // 2]
        in_maps.append({
            "xT": _pack_x(xr[c], np_mm),                             # [P, KT*T]
            "aT": aT,
            "m2x": m2x.astype(np_mm),
        })

    res = run_bass_kernel_spmd(_get_nc(), in_maps, core_ids=list(range(N_CORES)))

    out = np.empty((N_CORES, T, OUT), np.float32)
    for c in range(N_CORES):
        out[c] = _unpack_out(res.results[c]["outT"])
    return out.reshape(B, S, OUT)
